# revision 1
# baseline (speedup 1.0000x reference)
"""TRN2 Bass kernel for nn_Attention_20444044329649.

GroupNorm(32) -> qkv dense -> single-head spatial attention (1024 pos) ->
out dense -> residual.  B=32 examples sharded 4-per-core across 8 cores;
params replicated.

Layout strategy per example (N=1024 positions, C=512 channels):
  xN   [128p, 8i, 512c]  natural (DMA'd), used for transposes + residual
  xT   [128p, 4t, 1024i] channels-on-partitions (PE transposes)
  zT   = GN(xT)          fp32r
  qT,kT [128, 4t, 1024i] via matmul(lhsT=w_qkv mtile, rhs=zT)
  v    [128, 8i, 512c]   natural via matmul(lhsT=zT islice, rhs=w_v)
  ST   [j, i] scores transposed  (lhsT=kT, rhs=qT) -> exp -> ET fp32r
  s    [1, 1024] softmax denominators via ones-matmul; folded into the
       final residual add as a per-partition reciprocal scale
  OT   [128, 4c, 1024i] = v^T @ ET  (lhsT=v jslice, rhs=ET)
  out  [i, d] = OT^T @ w_out, then x + recip_s * out + b_out

Softmax runs without max subtraction: scores here are ~N(0,1) (bounded
well within fp32 exp range); result matches jax softmax to fp32 rounding.

All large matmuls run in fp32r (inputs rounded to 11-bit mantissa, fp32
accumulate; ~3.8x faster than fp32 on the PE).  Measured end-to-end
absmax error vs the fp32 reference: 4.1e-4 on outputs of magnitude ~5.3
(relative 7.5e-5).  Measured HW exec time: ~337 us per core.

Emission is software-pipelined across examples: the load/transpose/
stats/normalize pre-stage of example bi+1 is emitted between attention
phase A and phase B/C of example bi so the cross-engine stats chain
overlaps PE attention work.
"""

import numpy as np

import concourse.bass as bass
import concourse.mybir as mybir
import concourse.tile as tile
from concourse import bacc
from concourse.bass_utils import run_bass_kernel_spmd
from concourse.masks import make_identity

B, H, W, C = 32, 32, 32, 512
N = H * W                      # 1024 positions
G = 32                         # groups
GS = C // G                    # 16 channels per group
EPS = 1e-5
NCORES = 8
BPC = B // NCORES              # 4 examples per core
ISQ = float(1.0 / np.sqrt(C))  # score scale

F32 = mybir.dt.float32
F32R = mybir.dt.float32r
AF = mybir.ActivationFunctionType
ALU = mybir.AluOpType
MS = bass.MemorySpace

MM_DT = F32R                   # dtype for the big matmuls


class Ctx:
    pass


def _load_x(g, bi):
    xn = g.xn_p.tile([128, 8, 512], F32, tag="xn", name=f"xn{bi}")
    for d in range(8):
        eng = g.nc.sync if d % 2 == 0 else g.nc.scalar
        eng.dma_start(xn[:, d, :], g.xr[bi, :, d, :])
    return xn


def _pre_stage(g, bi, xn=None):
    if xn is None:
        xn = _load_x(g, bi)
    xt = _pre_transpose(g, bi, xn)
    zt = _pre_stats(g, bi, xt)
    return xn, zt


def _pre_transpose(g, bi, xn, copy_on_act=False):
    """Transpose x to channel-major xT via the PE."""
    nc = g.nc
    xt = g.xt_p.tile([128, 4, 1024], F32, tag="xt", name=f"xt{bi}")
    for t in range(4):
        for half in range(2):
            ps = g.pm.tile([128, 512], F32, tag="pm", name=f"ps_tr{bi}_{t}_{half}")
            for q in range(4):
                i = half * 4 + q
                nc.tensor.matmul(
                    ps[:, q * 128:(q + 1) * 128],
                    xn[:, i, t * 128:(t + 1) * 128],
                    g.ident,
                    is_transpose=True,
                    start=(q == 0),
                    stop=(q == 3),
                )
            if copy_on_act:
                nc.scalar.copy(xt[:, t, half * 512:(half + 1) * 512], ps)
            else:
                nc.vector.tensor_copy(xt[:, t, half * 512:(half + 1) * 512], ps)
    return xt


def _pre_stats(g, bi, xt):
    """Group-norm stats + normalize -> zT (fp32r)."""
    nc = g.nc
    zt = g.zt_p.tile([128, 4, 1024], MM_DT, tag="zt", name=f"zt{bi}")
    for t in range(4):
        st6 = g.small.tile([128, 2, 6], F32, tag="st6")
        for s in range(2):
            nc.vector.bn_stats(st6[:, s, :], xt[:, t, s * 512:(s + 1) * 512])
        mv = g.small.tile([128, 2], F32, tag="mv")
        nc.vector.bn_aggr(mv, st6)
        # m2 = [mean, E[x^2]] per channel
        m2 = g.small.tile([128, 2], F32, tag="m2")
        nc.vector.tensor_copy(m2[:, 0:1], mv[:, 0:1])
        nc.vector.tensor_mul(m2[:, 1:2], mv[:, 0:1], mv[:, 0:1])
        nc.vector.tensor_add(m2[:, 1:2], m2[:, 1:2], mv[:, 1:2])
        # pool over groups of 16 channels: [8, 2] = a_pool^T @ m2
        ps_g = g.aux.tile([8, 2], F32, tag="aux")
        nc.tensor.matmul(ps_g, g.a_pool, m2, start=True, stop=True)
        # per-group [rstd, mean]
        pg = g.small.tile([8, 2], F32, tag="pg")
        nc.vector.tensor_copy(pg, ps_g)
        gab = g.small.tile([8, 2], F32, tag="gab")
        tmp8 = g.small.tile([8, 1], F32, tag="tmp8")
        nc.vector.tensor_mul(tmp8, pg[:, 0:1], pg[:, 0:1])
        nc.vector.tensor_sub(gab[:, 0:1], pg[:, 1:2], tmp8)
        # rstd = 1/sqrt(var+eps), then one Newton step u*(1.5-0.5*y*u^2)
        # to clean up the Sqrt-table + reciprocal() low bits (all tiny ops)
        y8 = g.small.tile([8, 1], F32, tag="y8")
        nc.vector.tensor_scalar(out=y8, in0=gab[:, 0:1], scalar1=EPS,
                                scalar2=0.0, op0=ALU.add, op1=ALU.add)
        nc.scalar.activation(gab[:, 0:1], gab[:, 0:1], AF.Sqrt, bias=g.eps_c[:8])
        nc.vector.reciprocal(gab[:, 0:1], gab[:, 0:1])
        u1 = g.small.tile([8, 1], F32, tag="u1")
        nc.vector.tensor_mul(u1, gab[:, 0:1], gab[:, 0:1])
        nc.vector.tensor_mul(u1, u1, y8)
        nc.vector.tensor_scalar(out=u1, in0=u1, scalar1=-0.5, scalar2=1.5,
                                op0=ALU.mult, op1=ALU.add)
        nc.vector.tensor_mul(gab[:, 0:1], gab[:, 0:1], u1)
        nc.vector.tensor_copy(gab[:, 1:2], pg[:, 0:1])
        # expand to channels: [128, 2] = e8^T @ gab
        ps_ab = g.aux.tile([128, 2], F32, tag="aux")
        nc.tensor.matmul(ps_ab, g.e8, gab, start=True, stop=True)
        # A = rstd * gn_scale ; Bb = gn_bias - mean * A
        ab = g.small.tile([128, 2], F32, tag="ab")
        tmpc = g.small.tile([128, 1], F32, tag="tmpc")
        nc.vector.tensor_mul(ab[:, 0:1], ps_ab[:, 0:1], g.gns_sb[:, t:t + 1])
        nc.vector.tensor_mul(tmpc, ps_ab[:, 1:2], ab[:, 0:1])
        nc.vector.tensor_sub(ab[:, 1:2], g.gnb_sb[:, t:t + 1], tmpc)
        nc.vector.tensor_scalar(
            out=zt[:, t, :], in0=xt[:, t, :],
            scalar1=ab[:, 0:1], scalar2=ab[:, 1:2],
            op0=ALU.mult, op1=ALU.add,
        )
        if bi == 0:
            g.warm(2)  # keep the HAM window busy through the serial chain
    return zt


def _fast_reduce(g, bi, xn):
    """Per-chunk (position-row x group) sums of x and x^2, emitted right
    after the x DMAs so the DVE/ACT start while later chunks stream in."""
    nc = g.nc
    rb = g.small.tile([128, 8, 64], F32, tag="rb")
    for d in range(8):
        xv = xn[:, d, :].rearrange("p (gg c) -> p gg c", c=GS)
        nc.vector.reduce_sum(rb[:, d, 0:32], xv, axis=mybir.AxisListType.X)
        sq = g.small.tile([128, 32, 16], F32, tag="sq", name=f"sq{bi}_{d}")
        nc.scalar.activation(sq, xv, AF.Square)
        nc.vector.reduce_sum(rb[:, d, 32:64], sq, axis=mybir.AxisListType.X)
    g.rb = rb


def _pre_stats_fast(g, bi, xn, xt):
    """Group-norm stats computed directly from natural-layout xn chunks:
    per-chunk (position-row x group) partial sums on the DVE, pooled
    across partitions by a ones-matmul.  Shortens the example-0 critical
    chain (no dependency on the transpose PSUM->SBUF copies)."""
    nc = g.nc
    zt = g.zt_p.tile([128, 4, 1024], MM_DT, tag="zt", name=f"ztf{bi}")
    rb = g.rb
    s_g = g.aux.tile([1, 64], F32, tag="aux", name="s_g")
    for d in range(8):
        nc.tensor.matmul(s_g, g.ones_f, rb[:, d, :],
                         start=(d == 0), stop=(d == 7))
    sg_sb = g.small.tile([1, 64], F32, tag="sg_sb")
    nc.vector.tensor_copy(sg_sb, s_g)
    gst = g.small.tile([32, 2], F32, tag="gst")
    for j in range(2):
        trg = g.pm.tile([32, 1], F32, tag="pm", name=f"trg{j}")
        nc.tensor.matmul(trg, sg_sb[0:1, 32 * j:32 * (j + 1)],
                         g.ident[0:1, 0:1], is_transpose=True,
                         start=True, stop=True)
        nc.vector.tensor_copy(gst[:, j:j + 1], trg)
    me = g.small.tile([32, 2], F32, tag="me")
    nc.vector.tensor_scalar(out=me, in0=gst, scalar1=1.0 / (N * GS),
                            scalar2=0.0, op0=ALU.mult, op1=ALU.add)
    v32 = g.small.tile([32, 1], F32, tag="v32")
    nc.vector.tensor_mul(v32, me[:, 0:1], me[:, 0:1])
    nc.vector.tensor_sub(v32, me[:, 1:2], v32)
    y32 = g.small.tile([32, 1], F32, tag="y32")
    nc.vector.tensor_scalar(out=y32, in0=v32, scalar1=EPS, scalar2=0.0,
                            op0=ALU.add, op1=ALU.add)
    nc.scalar.activation(v32, v32, AF.Sqrt, bias=g.eps_c[:32])
    nc.vector.reciprocal(v32, v32)
    u32 = g.small.tile([32, 1], F32, tag="u32")
    nc.vector.tensor_mul(u32, v32, v32)
    nc.vector.tensor_mul(u32, u32, y32)
    nc.vector.tensor_scalar(out=u32, in0=u32, scalar1=-0.5, scalar2=1.5,
                            op0=ALU.mult, op1=ALU.add)
    nc.vector.tensor_mul(v32, v32, u32)
    gab32 = g.small.tile([32, 2], F32, tag="gab32")
    nc.vector.tensor_copy(gab32[:, 0:1], v32)
    nc.vector.tensor_copy(gab32[:, 1:2], me[:, 0:1])
    for t in range(4):
        ps_ab = g.pm.tile([128, 2], F32, tag="pm", name=f"ps_abf{t}")
        nc.tensor.matmul(ps_ab, g.e32[:, t * 128:(t + 1) * 128], gab32,
                         start=True, stop=True)
        ab = g.small.tile([128, 2], F32, tag="ab")
        tmpc = g.small.tile([128, 1], F32, tag="tmpc")
        nc.vector.tensor_mul(ab[:, 0:1], ps_ab[:, 0:1], g.gns_sb[:, t:t + 1])
        nc.vector.tensor_mul(tmpc, ps_ab[:, 1:2], ab[:, 0:1])
        nc.vector.tensor_sub(ab[:, 1:2], g.gnb_sb[:, t:t + 1], tmpc)
        nc.vector.tensor_scalar(
            out=zt[:, t, :], in0=xt[:, t, :],
            scalar1=ab[:, 0:1], scalar2=ab[:, 1:2],
            op0=ALU.mult, op1=ALU.add,
        )
        g.warm(2)
    return zt


def _qkv_stage(g, zt):
    nc = g.nc
    qt = g.qt_p.tile([128, 4, 1024], MM_DT, tag="qt")
    kt = g.kt_p.tile([128, 4, 1024], MM_DT, tag="kt")
    for which, dst in ((0, qt), (1, kt)):
        for m in range(4):
            for h in range(2):
                ps = g.pm.tile([128, 512], F32, tag="pm")
                for kk in range(4):
                    nc.tensor.matmul(
                        ps,
                        g.wqkv_sb[:, kk, which * 512 + m * 128:which * 512 + (m + 1) * 128],
                        zt[:, kk, h * 512:(h + 1) * 512],
                        start=(kk == 0),
                        stop=(kk == 3),
                    )
                nc.scalar.activation(
                    dst[:, m, h * 512:(h + 1) * 512], ps, AF.Identity,
                    bias=g.bq_sb[:, which * 4 + m:which * 4 + m + 1],
                )
    v = g.v_p.tile([128, 8, 512], MM_DT, tag="v")
    for i in range(8):
        ps = g.pm.tile([128, 512], F32, tag="pm")
        for kk in range(4):
            nc.tensor.matmul(
                ps,
                zt[:, kk, i * 128:(i + 1) * 128],
                g.wqkv_sb[:, kk, 1024:1536],
                start=(kk == 0),
                stop=(kk == 3),
            )
        nc.vector.tensor_add(v[:, i, :], ps, g.bv_bc)
    return qt, kt, v


def _phase_a(g, qt, kt):
    """Transposed scores + exp + softmax-denominator accumulation."""
    nc = g.nc
    et = g.et_p.tile([128, 8, 1024], MM_DT, tag="et")
    s_ps = [g.pm.tile([1, 512], F32, tag="pm", name=f"s_ps{h}") for h in range(2)]

    def ones_mm(j):
        for h in range(2):
            nc.tensor.matmul(
                s_ps[h], g.ones_r, et[:, j, h * 512:(h + 1) * 512],
                start=(j == 0), stop=(j == 7),
            )

    for j in range(8):
        for h in range(2):
            ps = g.pm.tile([128, 512], F32, tag="pm")
            for ct in range(4):
                nc.tensor.matmul(
                    ps,
                    kt[:, ct, j * 128:(j + 1) * 128],
                    qt[:, ct, h * 512:(h + 1) * 512],
                    start=(ct == 0),
                    stop=(ct == 3),
                )
            nc.scalar.activation(
                et[:, j, h * 512:(h + 1) * 512], ps, AF.Exp, scale=ISQ,
            )
        if j > 0:
            ones_mm(j - 1)  # pipelined: exp(j-1) finished while ST(j) ran
    ones_mm(7)
    return et, s_ps


def _phase_bc(g, bi, xn, v, et, s_ps):
    """Softmax denominators, O^T, out-projection, residual, store."""
    nc = g.nc
    # s -> SBUF -> DRAM bounce -> [128, 8] -> reciprocal
    s_sb = g.s_p.tile([1, 1024], F32, tag="s_sb")
    for h in range(2):
        nc.vector.tensor_copy(s_sb[:, h * 512:(h + 1) * 512], s_ps[h])
    s_dram = g.dram.tile([1, 1024], F32, tag="s_dram")
    nc.sync.dma_start(s_dram, s_sb)
    s_col = g.small.tile([128, 8], F32, tag="s_col")
    nc.sync.dma_start(s_col, s_dram.rearrange("o (t p) -> p (o t)", p=128))
    recip = g.small.tile([128, 8], F32, tag="recip")
    nc.vector.reciprocal(recip, s_col)
    # one Newton step: r <- r * (2 - s*r), fixes reciprocal() low bits
    rt1 = g.small.tile([128, 8], F32, tag="rt1")
    nc.vector.tensor_mul(rt1, s_col, recip)
    nc.vector.tensor_scalar(out=rt1, in0=rt1, scalar1=-1.0, scalar2=2.0,
                            op0=ALU.mult, op1=ALU.add)
    nc.vector.tensor_mul(recip, recip, rt1)

    # b_out pre-add into xn (after the transposes read xn)
    for i in range(8):
        nc.vector.tensor_add(xn[:, i, :], xn[:, i, :], g.bout_bc)

    ot = g.qt_p.tile([128, 4, 1024], MM_DT, tag="qt")    # reuses qt slot
    res = g.kt_p.tile([128, 8, 512], F32, tag="kt")      # reuses kt slot
    for h in range(2):
        ps_ot = [g.pm.tile([128, 512], F32, tag="pm", name=f"ps_ot{c}") for c in range(4)]
        for j in range(8):
            for ct in range(4):
                nc.tensor.matmul(
                    ps_ot[ct],
                    v[:, j, ct * 128:(ct + 1) * 128],
                    et[:, j, h * 512:(h + 1) * 512],
                    start=(j == 0),
                    stop=(j == 7),
                )
        for ct in range(4):
            nc.scalar.copy(ot[:, ct, h * 512:(h + 1) * 512], ps_ot[ct])
        for q in range(4):
            i = h * 4 + q
            psf = g.pm.tile([128, 512], F32, tag="pm")
            for ct in range(4):
                nc.tensor.matmul(
                    psf,
                    ot[:, ct, i * 128:(i + 1) * 128],
                    g.wout_sb[:, ct, :],
                    start=(ct == 0),
                    stop=(ct == 3),
                )
            nc.vector.scalar_tensor_tensor(
                out=res[:, i, :], in0=psf, scalar=recip[:, i:i + 1],
                in1=xn[:, i, :], op0=ALU.mult, op1=ALU.add,
            )
            nc.sync.dma_start(g.outr[bi, :, i, :], res[:, i, :])


def build_program():
    nc = bacc.Bacc("TRN2", target_bir_lowering=False, debug=False)

    x_d = nc.dram_tensor("x", [BPC, N, C], F32, kind="ExternalInput")
    wqkv_d = nc.dram_tensor("w_qkv", [C, 3 * C], F32, kind="ExternalInput")
    bqkv_d = nc.dram_tensor("b_qkv", [3 * C], F32, kind="ExternalInput")
    wout_d = nc.dram_tensor("w_out", [C, C], F32, kind="ExternalInput")
    bout_d = nc.dram_tensor("b_out", [C], F32, kind="ExternalInput")
    gns_d = nc.dram_tensor("gn_scale", [C], F32, kind="ExternalInput")
    gnb_d = nc.dram_tensor("gn_bias", [C], F32, kind="ExternalInput")
    out_d = nc.dram_tensor("out", [BPC, N, C], F32, kind="ExternalOutput")

    g = Ctx()
    g.nc = nc
    g.xr = x_d.ap().rearrange("b (i p) c -> b p i c", p=128)
    g.outr = out_d.ap().rearrange("b (i p) c -> b p i c", p=128)

    with tile.TileContext(nc) as tc:
        from contextlib import ExitStack
        with ExitStack() as ctx:
            const = ctx.enter_context(tc.tile_pool(name="const", bufs=1))
            g.pm = ctx.enter_context(tc.tile_pool(name="pm", bufs=7, space=MS.PSUM))
            g.aux = ctx.enter_context(tc.tile_pool(name="aux", bufs=1, space=MS.PSUM))
            g.dram = ctx.enter_context(tc.tile_pool(name="dram", bufs=2, space=MS.DRAM))
            g.xn_p = ctx.enter_context(tc.tile_pool(name="xn", bufs=2))
            g.xt_p = ctx.enter_context(tc.tile_pool(name="xtp", bufs=1))
            g.zt_p = ctx.enter_context(tc.tile_pool(name="ztp", bufs=1))
            g.qt_p = ctx.enter_context(tc.tile_pool(name="qtp", bufs=1))
            g.kt_p = ctx.enter_context(tc.tile_pool(name="ktp", bufs=1))
            g.v_p = ctx.enter_context(tc.tile_pool(name="vp", bufs=1))
            g.et_p = ctx.enter_context(tc.tile_pool(name="etp", bufs=1))
            g.small = ctx.enter_context(tc.tile_pool(name="small", bufs=3))
            g.s_p = ctx.enter_context(tc.tile_pool(name="s_p", bufs=1))

            # ---- example-0 input DMA first: it is on the critical path
            xn0 = _load_x(g, 0)

            # ---- constants ----------------------------------------------
            g.ident = const.tile([128, 128], F32)
            make_identity(nc, g.ident)

            # PE warmup: real matmuls with no DMA dependency, issued while
            # the input DMAs run, so the HAM clock gate reaches K=8/8
            # before the first productive matmul.
            def warm(n, salt=[0]):
                for _ in range(n):
                    salt[0] += 1
                    ps_w = g.pm.tile([128, 512], F32, tag="pm",
                                     name=f"ps_w{salt[0]}")
                    nc.tensor.matmul(ps_w[:, 0:128], g.ident, g.ident,
                                     start=True, stop=True)
            g.warm = warm
            warm(24)

            g.a_pool = const.tile([128, 8], F32)
            nc.gpsimd.memset(g.a_pool, 1.0 / GS)
            nc.gpsimd.affine_select(
                out=g.a_pool, in_=g.a_pool, compare_op=ALU.is_ge, fill=0.0,
                base=0, pattern=[[-GS, 8]], channel_multiplier=1)
            nc.gpsimd.affine_select(
                out=g.a_pool, in_=g.a_pool, compare_op=ALU.is_ge, fill=0.0,
                base=GS - 1, pattern=[[GS, 8]], channel_multiplier=-1)

            g.e8 = const.tile([8, 128], F32)
            nc.gpsimd.memset(g.e8, 1.0)
            nc.gpsimd.affine_select(
                out=g.e8, in_=g.e8, compare_op=ALU.is_ge, fill=0.0,
                base=0, pattern=[[1, 128]], channel_multiplier=-GS)
            nc.gpsimd.affine_select(
                out=g.e8, in_=g.e8, compare_op=ALU.is_ge, fill=0.0,
                base=GS - 1, pattern=[[-1, 128]], channel_multiplier=GS)

            ones_f = const.tile([128, 1], F32)
            nc.vector.memset(ones_f, 1.0)
            g.ones_f = ones_f
            g.ones_r = const.tile([128, 1], MM_DT)
            nc.gpsimd.dma_start(g.ones_r, ones_f)

            # E32[g, c] = 1 if c // 16 == g, for the example-0 fast-stats
            # group -> channel expansion
            g.e32 = const.tile([32, 512], F32)
            nc.gpsimd.memset(g.e32, 1.0)
            nc.gpsimd.affine_select(
                out=g.e32, in_=g.e32, compare_op=ALU.is_ge, fill=0.0,
                base=0, pattern=[[1, 512]], channel_multiplier=-GS)
            nc.gpsimd.affine_select(
                out=g.e32, in_=g.e32, compare_op=ALU.is_ge, fill=0.0,
                base=GS - 1, pattern=[[-1, 512]], channel_multiplier=GS)
            g.eps_c = const.tile([128, 1], F32)
            nc.vector.memset(g.eps_c, EPS)

            g.wqkv_sb = const.tile([128, 4, 3 * C], MM_DT)
            wqr = wqkv_d.ap().rearrange("(t p) d -> t p d", p=128)
            for t in range(4):
                nc.gpsimd.dma_start(g.wqkv_sb[:, t, :], wqr[t])
            g.wout_sb = const.tile([128, 4, C], MM_DT)
            wor = wout_d.ap().rearrange("(t p) d -> t p d", p=128)
            for t in range(4):
                nc.gpsimd.dma_start(g.wout_sb[:, t, :], wor[t])

            g.bq_sb = const.tile([128, 12], F32)
            nc.sync.dma_start(g.bq_sb, bqkv_d.ap().rearrange("(m p) -> p m", p=128))
            g.gns_sb = const.tile([128, 4], F32)
            nc.sync.dma_start(g.gns_sb, gns_d.ap().rearrange("(t p) -> p t", p=128))
            g.gnb_sb = const.tile([128, 4], F32)
            nc.sync.dma_start(g.gnb_sb, gnb_d.ap().rearrange("(t p) -> p t", p=128))

            def bcast(src_ap):
                return bass.AP(
                    tensor=src_ap.tensor, offset=src_ap.offset,
                    ap=[[0, 128]] + [list(p) for p in src_ap.ap])

            g.bv_bc = const.tile([128, 512], F32)
            nc.gpsimd.dma_start(g.bv_bc, bcast(bqkv_d.ap()[2 * C:3 * C]))
            g.bout_bc = const.tile([128, 512], F32)
            nc.gpsimd.dma_start(g.bout_bc, bcast(bout_d.ap()))

            # ---- pipelined per-example emission -------------------------
            _fast_reduce(g, 0, xn0)
            xt0 = _pre_transpose(g, 0, xn0)
            zt0 = _pre_stats_fast(g, 0, xn0, xt0)
            state = (xn0, zt0)
            for bi in range(BPC):
                xn, zt = state
                qt, kt, v = _qkv_stage(g, zt)
                et, s_ps = _phase_a(g, qt, kt)
                if bi + 1 < BPC:
                    state = _pre_stage(g, bi + 1)
                _phase_bc(g, bi, xn, v, et, s_ps)

    nc.compile()
    return nc


_NC = None


def _get_nc():
    global _NC
    if _NC is None:
        _NC = build_program()
    return _NC


def kernel(x, t, gn_scale, gn_bias, w_qkv, b_qkv, w_out, b_out):
    x = np.ascontiguousarray(np.asarray(x, np.float32).reshape(B, N, C))
    shared = {
        "w_qkv": np.ascontiguousarray(np.asarray(w_qkv, np.float32)),
        "b_qkv": np.ascontiguousarray(np.asarray(b_qkv, np.float32)),
        "w_out": np.ascontiguousarray(np.asarray(w_out, np.float32)),
        "b_out": np.ascontiguousarray(np.asarray(b_out, np.float32)),
        "gn_scale": np.ascontiguousarray(np.asarray(gn_scale, np.float32)),
        "gn_bias": np.ascontiguousarray(np.asarray(gn_bias, np.float32)),
    }
    in_maps = [
        {"x": x[c * BPC:(c + 1) * BPC], **shared} for c in range(NCORES)
    ]
    nc = _get_nc()
    res = run_bass_kernel_spmd(nc, in_maps, core_ids=list(range(NCORES)))
    out = np.concatenate([res.results[c]["out"] for c in range(NCORES)], axis=0)
    return out.reshape(B, H, W, C)



# revision 2
# speedup vs baseline: 1.2573x; 1.2573x over previous
"""TRN2 Bass kernel for nn_Attention_20444044329649.

GroupNorm(32) -> qkv dense -> single-head spatial attention (1024 pos) ->
out dense -> residual.  B=32 examples sharded 4-per-core across 8 cores;
params replicated.

v2 — algebraic folds that cut PE matmul work ~35% vs v1:

  * scores:  S*isq = Z (isq*Wq Wk^T) Z^T = Z M' Z^T.  M' is host-
    precomputed, so the device computes only G^T = M'^T Z^T (32 MMs) and
    then ST = matmul(lhsT=zT, rhs=G^T) — the K projection disappears.
    The q/k bias terms add a per-query constant (cancels in softmax) and
    a per-key term u_j = (Wk bq)·z_j * isq, applied as a per-partition
    bias in the exp activation (only emitted when b_qkv != 0).
  * out-proj: (A V) W_out = A (V W_out).  Wv' = Wv W_out is host-folded,
    b' = bv W_out + b_out goes into the residual pre-add.  O is computed
    in NATURAL layout via lhsT=ET chunks, rhs=V' — the out projection
    and the O^T->O transposes disappear.
  * softmax denominators: N=1 matmuls (lhsT=ET chunk, rhs=ones)
    interleaved with the O matmuls (sharing stationary weights), landing
    s[i] per-partition exactly where the residual scale needs it.  The
    16 N=512 ones-matmuls and the DRAM transpose bounce disappear.
  * big matmul operands in bf16 (same PE rate as fp32r; half the
    SBUF/evac bandwidth).  Accumulation stays fp32 in PSUM.  Residual
    path (xn) stays fp32 end-to-end.

Layout per example (N=1024 positions, C=512 channels):
  xN   [128p, 8i, 512c]  natural fp32 (DMA'd), residual + stats
  xT   [128p, 4t, 1024i] bf16, channels-on-partitions (PE transposes)
  zT   = GN(xT)          bf16
  gT   [128, 4t, 1024i]  = M'^T zT  (bf16)
  ST   [j, i] scores transposed (lhsT=zT, rhs=gT) -> exp -> ET bf16
  v    [128, 8i, 512c]   = Z Wv'  natural via matmul(lhsT=zT, rhs=Wv')
  O    [i, c] natural = matmul(lhsT=ET islice, rhs=v), accum over j;
       denominators from interleaved N=1 matmuls vs ones
  out  x + recip_s * O + b'  (per-partition recip scale)

Emission is software-pipelined across examples as in v1.
"""

import numpy as np

import concourse.bass as bass
import concourse.mybir as mybir
import concourse.tile as tile
from concourse import bacc
from concourse.bass_utils import run_bass_kernel_spmd
from concourse.masks import make_identity

B, H, W, C = 32, 32, 32, 512
N = H * W                      # 1024 positions
G = 32                         # groups
GS = C // G                    # 16 channels per group
EPS = 1e-5
NCORES = 8
BPC = B // NCORES              # 4 examples per core
ISQ = float(1.0 / np.sqrt(C))  # score scale (folded into M' on host)

F32 = mybir.dt.float32
BF16 = mybir.dt.bfloat16
AF = mybir.ActivationFunctionType
ALU = mybir.AluOpType
MS = bass.MemorySpace

MM_DT = BF16                   # dtype for the big matmul operands


class Ctx:
    pass


def _load_x(g, bi):
    xn = g.xn_p.tile([128, 8, 512], F32, tag="xn", name=f"xn{bi}")
    for d in range(8):
        eng = g.nc.sync if d % 2 == 0 else g.nc.scalar
        eng.dma_start(xn[:, d, :], g.xr[bi, :, d, :])
    return xn


def _pre_stage(g, bi, xn=None):
    if xn is None:
        xn = _load_x(g, bi)
    xt = _pre_transpose(g, bi, xn)
    zt = _pre_stats(g, bi, xt)
    return xn, zt


def _pre_transpose(g, bi, xn):
    """Transpose x to channel-major xT (bf16) via the PE."""
    nc = g.nc
    xt = g.xt_p.tile([128, 4, 1024], MM_DT, tag="xt", name=f"xt{bi}")
    for t in range(4):
        for half in range(2):
            ps = g.pm.tile([128, 512], F32, tag="pm", name=f"ps_tr{bi}_{t}_{half}")
            for q in range(4):
                i = half * 4 + q
                nc.tensor.matmul(
                    ps[:, q * 128:(q + 1) * 128],
                    xn[:, i, t * 128:(t + 1) * 128],
                    g.ident,
                    is_transpose=True,
                    start=(q == 0),
                    stop=(q == 3),
                )
            nc.vector.tensor_copy(xt[:, t, half * 512:(half + 1) * 512], ps)
    return xt


def _pre_stats(g, bi, xt):
    """Group-norm stats + normalize -> zT (bf16)."""
    nc = g.nc
    zt = g.zt_p.tile([128, 4, 1024], MM_DT, tag="zt", name=f"zt{bi}")
    for t in range(4):
        st6 = g.small.tile([128, 2, 6], F32, tag="st6")
        for s in range(2):
            nc.vector.bn_stats(st6[:, s, :], xt[:, t, s * 512:(s + 1) * 512])
        mv = g.small.tile([128, 2], F32, tag="mv")
        nc.vector.bn_aggr(mv, st6)
        # m2 = [mean, E[x^2]] per channel
        m2 = g.small.tile([128, 2], F32, tag="m2")
        nc.vector.tensor_copy(m2[:, 0:1], mv[:, 0:1])
        nc.vector.tensor_mul(m2[:, 1:2], mv[:, 0:1], mv[:, 0:1])
        nc.vector.tensor_add(m2[:, 1:2], m2[:, 1:2], mv[:, 1:2])
        # pool over groups of 16 channels: [8, 2] = a_pool^T @ m2
        ps_g = g.aux.tile([8, 2], F32, tag="aux")
        nc.tensor.matmul(ps_g, g.a_pool, m2, start=True, stop=True)
        # per-group [rstd, mean]
        pg = g.small.tile([8, 2], F32, tag="pg")
        nc.vector.tensor_copy(pg, ps_g)
        gab = g.small.tile([8, 2], F32, tag="gab")
        tmp8 = g.small.tile([8, 1], F32, tag="tmp8")
        nc.vector.tensor_mul(tmp8, pg[:, 0:1], pg[:, 0:1])
        nc.vector.tensor_sub(gab[:, 0:1], pg[:, 1:2], tmp8)
        nc.scalar.activation(gab[:, 0:1], gab[:, 0:1], AF.Sqrt, bias=g.eps_c[:8])
        nc.vector.reciprocal(gab[:, 0:1], gab[:, 0:1])
        nc.vector.tensor_copy(gab[:, 1:2], pg[:, 0:1])
        # expand to channels: [128, 2] = e8^T @ gab
        ps_ab = g.aux.tile([128, 2], F32, tag="aux")
        nc.tensor.matmul(ps_ab, g.e8, gab, start=True, stop=True)
        # A = rstd * gn_scale ; Bb = gn_bias - mean * A
        ab = g.small.tile([128, 2], F32, tag="ab")
        tmpc = g.small.tile([128, 1], F32, tag="tmpc")
        nc.vector.tensor_mul(ab[:, 0:1], ps_ab[:, 0:1], g.gns_sb[:, t:t + 1])
        nc.vector.tensor_mul(tmpc, ps_ab[:, 1:2], ab[:, 0:1])
        nc.vector.tensor_sub(ab[:, 1:2], g.gnb_sb[:, t:t + 1], tmpc)
        nc.vector.tensor_scalar(
            out=zt[:, t, :], in0=xt[:, t, :],
            scalar1=ab[:, 0:1], scalar2=ab[:, 1:2],
            op0=ALU.mult, op1=ALU.add,
        )
    return zt


def _fast_reduce(g, bi, xn):
    """Per-chunk (position-row x group) sums of x and x^2, emitted right
    after the x DMAs so the DVE/ACT start while later chunks stream in."""
    nc = g.nc
    rb = g.small.tile([128, 8, 64], F32, tag="rb")
    for d in range(8):
        xv = xn[:, d, :].rearrange("p (gg c) -> p gg c", c=GS)
        nc.vector.reduce_sum(rb[:, d, 0:32], xv, axis=mybir.AxisListType.X)
        sq = g.small.tile([128, 32, 16], F32, tag="sq", name=f"sq{bi}_{d}")
        nc.scalar.activation(sq, xv, AF.Square)
        nc.vector.reduce_sum(rb[:, d, 32:64], sq, axis=mybir.AxisListType.X)
    g.rb = rb


def _pre_stats_fast(g, bi, xn, xt):
    """Group-norm stats computed directly from natural-layout xn chunks:
    shortens the example-0 critical chain (no dependency on the
    transpose PSUM->SBUF copies)."""
    nc = g.nc
    zt = g.zt_p.tile([128, 4, 1024], MM_DT, tag="zt", name=f"ztf{bi}")
    rb = g.rb
    s_g = g.aux.tile([1, 64], F32, tag="aux", name="s_g")
    for d in range(8):
        nc.tensor.matmul(s_g, g.ones_f, rb[:, d, :],
                         start=(d == 0), stop=(d == 7))
    sg_sb = g.small.tile([1, 64], F32, tag="sg_sb")
    nc.vector.tensor_copy(sg_sb, s_g)
    gst = g.small.tile([32, 2], F32, tag="gst")
    for j in range(2):
        trg = g.pm.tile([32, 1], F32, tag="pm", name=f"trg{j}")
        nc.tensor.matmul(trg, sg_sb[0:1, 32 * j:32 * (j + 1)],
                         g.ident[0:1, 0:1], is_transpose=True,
                         start=True, stop=True)
        nc.vector.tensor_copy(gst[:, j:j + 1], trg)
    me = g.small.tile([32, 2], F32, tag="me")
    nc.vector.tensor_scalar(out=me, in0=gst, scalar1=1.0 / (N * GS),
                            scalar2=0.0, op0=ALU.mult, op1=ALU.add)
    v32 = g.small.tile([32, 1], F32, tag="v32")
    nc.vector.tensor_mul(v32, me[:, 0:1], me[:, 0:1])
    nc.vector.tensor_sub(v32, me[:, 1:2], v32)
    nc.scalar.activation(v32, v32, AF.Sqrt, bias=g.eps_c[:32])
    nc.vector.reciprocal(v32, v32)
    gab32 = g.small.tile([32, 2], F32, tag="gab32")
    nc.vector.tensor_copy(gab32[:, 0:1], v32)
    nc.vector.tensor_copy(gab32[:, 1:2], me[:, 0:1])
    for t in range(4):
        ps_ab = g.pm.tile([128, 2], F32, tag="pm", name=f"ps_abf{t}")
        nc.tensor.matmul(ps_ab, g.e32[:, t * 128:(t + 1) * 128], gab32,
                         start=True, stop=True)
        ab = g.small.tile([128, 2], F32, tag="ab")
        tmpc = g.small.tile([128, 1], F32, tag="tmpc")
        nc.vector.tensor_mul(ab[:, 0:1], ps_ab[:, 0:1], g.gns_sb[:, t:t + 1])
        nc.vector.tensor_mul(tmpc, ps_ab[:, 1:2], ab[:, 0:1])
        nc.vector.tensor_sub(ab[:, 1:2], g.gnb_sb[:, t:t + 1], tmpc)
        nc.vector.tensor_scalar(
            out=zt[:, t, :], in0=xt[:, t, :],
            scalar1=ab[:, 0:1], scalar2=ab[:, 1:2],
            op0=ALU.mult, op1=ALU.add,
        )
        g.warm(2)
    return zt


def _gv_stage(g, bi, zt):
    """G^T = M'^T Z^T and V' = Z Wv' (natural)."""
    nc = g.nc
    gt = g.gt_p.tile([128, 4, 1024], MM_DT, tag="gt")
    for m in range(4):
        ps = [g.pm.tile([128, 512], F32, tag="pm", name=f"ps_g{bi}_{m}_{h}")
              for h in range(2)]
        for kk in range(4):
            for h in range(2):
                nc.tensor.matmul(
                    ps[h],
                    g.mq_sb[:, kk, m * 128:(m + 1) * 128],
                    zt[:, kk, h * 512:(h + 1) * 512],
                    start=(kk == 0),
                    stop=(kk == 3),
                )
        for h in range(2):
            nc.scalar.copy(gt[:, m, h * 512:(h + 1) * 512], ps[h])
    v = g.v_p.tile([128, 8, 512], MM_DT, tag="v")
    for i in range(8):
        ps = g.pm.tile([128, 512], F32, tag="pm")
        for kk in range(4):
            nc.tensor.matmul(
                ps,
                zt[:, kk, i * 128:(i + 1) * 128],
                g.wvp_sb[:, kk, :],
                start=(kk == 0),
                stop=(kk == 3),
            )
        nc.vector.tensor_copy(v[:, i, :], ps)
    return gt, v


def _u_stage(g, bi, zt):
    """Per-key bias u_j = uvec . z_j  (only when b_qkv != 0)."""
    nc = g.nc
    ps_u = g.aux.tile([128, 8], F32, tag="aux", name=f"ps_u{bi}")
    for j in range(8):
        for kk in range(4):
            nc.tensor.matmul(
                ps_u[:, j:j + 1],
                zt[:, kk, j * 128:(j + 1) * 128],
                g.uv_sb[:, kk:kk + 1],
                start=(kk == 0),
                stop=(kk == 3),
            )
    u_sb = g.small.tile([128, 8], F32, tag="u_sb", name=f"u_sb{bi}")
    nc.vector.tensor_copy(u_sb, ps_u)
    return u_sb


def _phase_st(g, bi, zt, gt, u_sb=None):
    """Transposed scores + exp -> ET (bf16)."""
    nc = g.nc
    et = g.et_p.tile([128, 8, 1024], MM_DT, tag="et")
    for j in range(8):
        ps = [g.pm.tile([128, 512], F32, tag="pm", name=f"ps_s{bi}_{j}_{h}")
              for h in range(2)]
        for ct in range(4):
            for h in range(2):
                nc.tensor.matmul(
                    ps[h],
                    zt[:, ct, j * 128:(j + 1) * 128],
                    gt[:, ct, h * 512:(h + 1) * 512],
                    start=(ct == 0),
                    stop=(ct == 3),
                )
        for h in range(2):
            if u_sb is None:
                nc.scalar.activation(
                    et[:, j, h * 512:(h + 1) * 512], ps[h], AF.Exp)
            else:
                nc.scalar.activation(
                    et[:, j, h * 512:(h + 1) * 512], ps[h], AF.Exp,
                    bias=u_sb[:, j:j + 1])
    return et


def _phase_o(g, bi, xn, v, et):
    """O natural (+ denominators), residual, store."""
    nc = g.nc
    # b' pre-add into xn (after the transposes/stats read xn)
    for i in range(8):
        nc.vector.tensor_add(xn[:, i, :], xn[:, i, :], g.bres_bc)
    res = g.res_p.tile([128, 8, 512], F32, tag="res")
    for i in range(8):
        ps_o = g.pm.tile([128, 512], F32, tag="pm", name=f"ps_o{bi}_{i}")
        ps_s = g.aux.tile([128, 1], F32, tag="aux", name=f"ps_sd{bi}_{i}")
        for j in range(8):
            nc.tensor.matmul(
                ps_o,
                et[:, j, i * 128:(i + 1) * 128],
                v[:, j, :],
                start=(j == 0),
                stop=(j == 7),
            )
            nc.tensor.matmul(
                ps_s,
                et[:, j, i * 128:(i + 1) * 128],
                g.ones_r,
                start=(j == 0),
                stop=(j == 7),
            )
        rc = g.small.tile([128, 1], F32, tag="rc", name=f"rc{bi}_{i}")
        nc.vector.reciprocal(rc, ps_s)
        nc.vector.scalar_tensor_tensor(
            out=res[:, i, :], in0=ps_o, scalar=rc,
            in1=xn[:, i, :], op0=ALU.mult, op1=ALU.add,
        )
        nc.sync.dma_start(g.outr[bi, :, i, :], res[:, i, :])


def build_program(has_u):
    nc = bacc.Bacc("TRN2", target_bir_lowering=False, debug=False)

    x_d = nc.dram_tensor("x", [BPC, N, C], F32, kind="ExternalInput")
    mq_d = nc.dram_tensor("m_qk", [C, C], MM_DT, kind="ExternalInput")
    wvp_d = nc.dram_tensor("w_vp", [C, C], MM_DT, kind="ExternalInput")
    bres_d = nc.dram_tensor("b_res", [C], F32, kind="ExternalInput")
    gns_d = nc.dram_tensor("gn_scale", [C], F32, kind="ExternalInput")
    gnb_d = nc.dram_tensor("gn_bias", [C], F32, kind="ExternalInput")
    if has_u:
        uv_d = nc.dram_tensor("u_vec", [C], F32, kind="ExternalInput")
    out_d = nc.dram_tensor("out", [BPC, N, C], F32, kind="ExternalOutput")

    g = Ctx()
    g.nc = nc
    g.xr = x_d.ap().rearrange("b (i p) c -> b p i c", p=128)
    g.outr = out_d.ap().rearrange("b (i p) c -> b p i c", p=128)

    with tile.TileContext(nc) as tc:
        from contextlib import ExitStack
        with ExitStack() as ctx:
            const = ctx.enter_context(tc.tile_pool(name="const", bufs=1))
            g.pm = ctx.enter_context(tc.tile_pool(name="pm", bufs=6, space=MS.PSUM))
            g.aux = ctx.enter_context(tc.tile_pool(name="aux", bufs=2, space=MS.PSUM))
            g.xn_p = ctx.enter_context(tc.tile_pool(name="xn", bufs=2))
            g.xt_p = ctx.enter_context(tc.tile_pool(name="xtp", bufs=1))
            g.zt_p = ctx.enter_context(tc.tile_pool(name="ztp", bufs=1))
            g.gt_p = ctx.enter_context(tc.tile_pool(name="gtp", bufs=1))
            g.v_p = ctx.enter_context(tc.tile_pool(name="vp", bufs=1))
            g.et_p = ctx.enter_context(tc.tile_pool(name="etp", bufs=1))
            g.res_p = ctx.enter_context(tc.tile_pool(name="resp", bufs=1))
            g.small = ctx.enter_context(tc.tile_pool(name="small", bufs=3))

            # ---- example-0 input DMA first: it is on the critical path
            xn0 = _load_x(g, 0)

            # ---- constants ----------------------------------------------
            g.ident = const.tile([128, 128], F32)
            make_identity(nc, g.ident)

            # PE warmup: real matmuls with no DMA dependency, issued while
            # the input DMAs run, so the HAM clock gate reaches K=8/8
            # before the first productive matmul.
            def warm(n, salt=[0]):
                for _ in range(n):
                    salt[0] += 1
                    ps_w = g.pm.tile([128, 512], F32, tag="pm",
                                     name=f"ps_w{salt[0]}")
                    nc.tensor.matmul(ps_w[:, 0:128], g.ident, g.ident,
                                     start=True, stop=True)
            g.warm = warm
            warm(24)

            g.a_pool = const.tile([128, 8], F32)
            nc.gpsimd.memset(g.a_pool, 1.0 / GS)
            nc.gpsimd.affine_select(
                out=g.a_pool, in_=g.a_pool, compare_op=ALU.is_ge, fill=0.0,
                base=0, pattern=[[-GS, 8]], channel_multiplier=1)
            nc.gpsimd.affine_select(
                out=g.a_pool, in_=g.a_pool, compare_op=ALU.is_ge, fill=0.0,
                base=GS - 1, pattern=[[GS, 8]], channel_multiplier=-1)

            g.e8 = const.tile([8, 128], F32)
            nc.gpsimd.memset(g.e8, 1.0)
            nc.gpsimd.affine_select(
                out=g.e8, in_=g.e8, compare_op=ALU.is_ge, fill=0.0,
                base=0, pattern=[[1, 128]], channel_multiplier=-GS)
            nc.gpsimd.affine_select(
                out=g.e8, in_=g.e8, compare_op=ALU.is_ge, fill=0.0,
                base=GS - 1, pattern=[[-1, 128]], channel_multiplier=GS)

            ones_f = const.tile([128, 1], F32)
            nc.vector.memset(ones_f, 1.0)
            g.ones_f = ones_f
            g.ones_r = const.tile([128, 1], MM_DT)
            nc.vector.memset(g.ones_r, 1.0)

            # E32[g, c] = 1 if c // 16 == g, for the example-0 fast-stats
            # group -> channel expansion
            g.e32 = const.tile([32, 512], F32)
            nc.gpsimd.memset(g.e32, 1.0)
            nc.gpsimd.affine_select(
                out=g.e32, in_=g.e32, compare_op=ALU.is_ge, fill=0.0,
                base=0, pattern=[[1, 512]], channel_multiplier=-GS)
            nc.gpsimd.affine_select(
                out=g.e32, in_=g.e32, compare_op=ALU.is_ge, fill=0.0,
                base=GS - 1, pattern=[[-1, 512]], channel_multiplier=GS)
            g.eps_c = const.tile([128, 1], F32)
            nc.vector.memset(g.eps_c, EPS)

            g.mq_sb = const.tile([128, 4, C], MM_DT)
            mqr = mq_d.ap().rearrange("(t p) d -> t p d", p=128)
            for t in range(4):
                nc.gpsimd.dma_start(g.mq_sb[:, t, :], mqr[t])
            g.wvp_sb = const.tile([128, 4, C], MM_DT)
            wvr = wvp_d.ap().rearrange("(t p) d -> t p d", p=128)
            for t in range(4):
                nc.gpsimd.dma_start(g.wvp_sb[:, t, :], wvr[t])

            g.gns_sb = const.tile([128, 4], F32)
            nc.sync.dma_start(g.gns_sb, gns_d.ap().rearrange("(t p) -> p t", p=128))
            g.gnb_sb = const.tile([128, 4], F32)
            nc.sync.dma_start(g.gnb_sb, gnb_d.ap().rearrange("(t p) -> p t", p=128))

            def bcast(src_ap):
                return bass.AP(
                    tensor=src_ap.tensor, offset=src_ap.offset,
                    ap=[[0, 128]] + [list(p) for p in src_ap.ap])

            g.bres_bc = const.tile([128, 512], F32)
            nc.gpsimd.dma_start(g.bres_bc, bcast(bres_d.ap()))

            if has_u:
                uvf = const.tile([128, 4], F32)
                nc.sync.dma_start(uvf, uv_d.ap().rearrange("(t p) -> p t", p=128))
                g.uv_sb = const.tile([128, 4], MM_DT)
                nc.vector.tensor_copy(g.uv_sb, uvf)

            # ---- pipelined per-example emission -------------------------
            _fast_reduce(g, 0, xn0)
            xt0 = _pre_transpose(g, 0, xn0)
            zt0 = _pre_stats_fast(g, 0, xn0, xt0)
            state = (xn0, zt0)
            for bi in range(BPC):
                xn, zt = state
                gt, v = _gv_stage(g, bi, zt)
                u_sb = _u_stage(g, bi, zt) if has_u else None
                et = _phase_st(g, bi, zt, gt, u_sb)
                if bi + 1 < BPC:
                    state = _pre_stage(g, bi + 1)
                _phase_o(g, bi, xn, v, et)

    nc.compile()
    return nc


_NC = {}


def _get_nc(has_u):
    if has_u not in _NC:
        _NC[has_u] = build_program(has_u)
    return _NC[has_u]


def kernel(x, t, gn_scale, gn_bias, w_qkv, b_qkv, w_out, b_out):
    import ml_dtypes
    x = np.ascontiguousarray(np.asarray(x, np.float32).reshape(B, N, C))
    w_qkv = np.asarray(w_qkv, np.float32)
    b_qkv = np.asarray(b_qkv, np.float32)
    w_out = np.asarray(w_out, np.float32)
    b_out = np.asarray(b_out, np.float32)
    wq, wk, wv = w_qkv[:, 0:C], w_qkv[:, C:2 * C], w_qkv[:, 2 * C:3 * C]
    bq, bv = b_qkv[0:C], b_qkv[2 * C:3 * C]

    m_qk = ((wq @ wk.T) * ISQ).astype(ml_dtypes.bfloat16)
    w_vp = (wv @ w_out).astype(ml_dtypes.bfloat16)
    b_res = (bv @ w_out + b_out).astype(np.float32)
    u_vec = ((wk @ bq) * ISQ).astype(np.float32)
    has_u = bool(np.any(u_vec != 0.0))

    shared = {
        "m_qk": np.ascontiguousarray(m_qk),
        "w_vp": np.ascontiguousarray(w_vp),
        "b_res": np.ascontiguousarray(b_res),
        "gn_scale": np.ascontiguousarray(np.asarray(gn_scale, np.float32)),
        "gn_bias": np.ascontiguousarray(np.asarray(gn_bias, np.float32)),
    }
    if has_u:
        shared["u_vec"] = np.ascontiguousarray(u_vec)
    in_maps = [
        {"x": x[c * BPC:(c + 1) * BPC], **shared} for c in range(NCORES)
    ]
    nc = _get_nc(has_u)
    res = run_bass_kernel_spmd(nc, in_maps, core_ids=list(range(NCORES)))
    out = np.concatenate([res.results[c]["out"] for c in range(NCORES)], axis=0)
    return out.reshape(B, H, W, C)


# revision 5
# speedup vs baseline: 1.4368x; 1.1428x over previous
"""TRN2 Bass kernel for nn_Attention_20444044329649.

GroupNorm(32) -> qkv dense -> single-head spatial attention (1024 pos) ->
out dense -> residual.  B=32 examples sharded 4-per-core across 8 cores;
params replicated.

v3 — v2's algebraic folds plus PE-FIFO discipline:

  * scores:  S*isq = Z M' Z^T with M' = isq*Wq Wk^T host-precomputed;
    device computes G^T = M'^T Z^T only (no K projection).  q/k biases:
    per-query term cancels in softmax; per-key term u_j applied as a
    per-partition exp bias (only emitted when b_qkv != 0).
  * out-proj fold: Wv' = Wv W_out, b' = bv W_out + b_out; O computed in
    NATURAL layout via lhsT=ET chunks, rhs=V'.
  * softmax denominators: N=1 matmuls interleaved with the O matmuls.
  * big matmul operands bf16; accumulation fp32 in PSUM; residual fp32.
  * GroupNorm stats batched to exactly TWO tiny PE matmuls per example
    (pool + expand across all 4 channel chunks at once), emitted inside
    the PREVIOUS example's O phase where their DVE inputs are already
    complete — the PE strict-FIFO queue never waits on the serial DVE
    stats chain.
  * next example's x transposes interleaved into the ST j-loop: no-dep
    PE work that keeps the HAM activity window busy (transpose-mode gaps
    plus boundary stalls previously re-throttled the PE to 1.2 GHz for
    ~10 us every example).
  * zt/v/et double-buffered so cross-example WAR hazards never
    serialize; example-0 input DMA spread over 4 queues.
"""

import numpy as np

import concourse.bass as bass
import concourse.mybir as mybir
import concourse.tile as tile
from concourse import bacc
from concourse.bass_utils import run_bass_kernel_spmd
from concourse.masks import make_identity

B, H, W, C = 32, 32, 32, 512
N = H * W                      # 1024 positions
G = 32                         # groups
GS = C // G                    # 16 channels per group
EPS = 1e-5
NCORES = 8
BPC = B // NCORES              # 4 examples per core
ISQ = float(1.0 / np.sqrt(C))  # score scale (folded into M' on host)

F32 = mybir.dt.float32
BF16 = mybir.dt.bfloat16
AF = mybir.ActivationFunctionType
ALU = mybir.AluOpType
MS = bass.MemorySpace

MM_DT = BF16                   # dtype for the big matmul operands


class Ctx:
    pass


def _load_x(g, bi, nq=2):
    xn = g.xn_p.tile([128, 8, 512], F32, tag="xn", name=f"xn{bi}")
    qs = [g.nc.sync, g.nc.scalar, g.nc.gpsimd][:nq]
    for d in range(8):
        qs[d % nq].dma_start(xn[:, d, :], g.xr[bi, :, d, :])
    return xn


def _tr_group(g, bi, xn, xt, st6, t, half):
    """One transpose group: 4 PE transpose MMs -> PSUM -> xT copy -> stats."""
    nc = g.nc
    ps = g.pm.tile([128, 512], F32, tag="pm", name=f"ps_tr{bi}_{t}_{half}")
    for q in range(4):
        i = half * 4 + q
        nc.tensor.matmul(
            ps[:, q * 128:(q + 1) * 128],
            xn[:, i, t * 128:(t + 1) * 128],
            g.ident,
            is_transpose=True,
            start=(q == 0),
            stop=(q == 3),
        )
    nc.vector.tensor_copy(xt[:, t, half * 512:(half + 1) * 512], ps)
    nc.vector.bn_stats(st6[:, t, half, :], xt[:, t, half * 512:(half + 1) * 512])


def _stats_pool(g, bi, st6):
    """Aggregate bn stats, pool over the 16-channel groups (ONE tiny PE
    matmul), and produce per-group [rstd, mean] on 8 partitions."""
    nc = g.nc
    mv = g.small.tile([128, 4, 2], F32, tag="mv", name=f"mv{bi}")
    for t in range(4):
        nc.vector.bn_aggr(mv[:, t, :], st6[:, t, :, :])
    # m2 = [mean, E[x^2]] per channel x chunk
    m2 = g.small.tile([128, 4, 2], F32, tag="m2", name=f"m2{bi}")
    nc.vector.tensor_copy(m2[:, :, 0:1], mv[:, :, 0:1])
    nc.vector.tensor_mul(m2[:, :, 1:2], mv[:, :, 0:1], mv[:, :, 0:1])
    nc.vector.tensor_add(m2[:, :, 1:2], m2[:, :, 1:2], mv[:, :, 1:2])
    ps_g = g.aux.tile([8, 4, 2], F32, tag="aux", name=f"ps_g{bi}")
    nc.tensor.matmul(ps_g, g.a_pool, m2, start=True, stop=True)
    pg = g.small.tile([8, 4, 2], F32, tag="pg", name=f"pg{bi}")
    nc.vector.tensor_copy(pg, ps_g)
    vr = g.small.tile([8, 4, 1], F32, tag="vr", name=f"vr{bi}")
    nc.vector.tensor_mul(vr, pg[:, :, 0:1], pg[:, :, 0:1])
    nc.vector.tensor_sub(vr, pg[:, :, 1:2], vr)
    nc.scalar.activation(vr, vr, AF.Sqrt, bias=g.eps_c[:8])
    nc.vector.reciprocal(vr, vr)
    gab = g.small.tile([8, 4, 2], F32, tag="gab", name=f"gab{bi}")
    nc.vector.tensor_copy(gab[:, :, 0:1], vr)
    nc.vector.tensor_copy(gab[:, :, 1:2], pg[:, :, 0:1])
    return gab


def _stats_norm(g, bi, xt, gab):
    """Expand group stats to channels (ONE tiny PE matmul) + normalize."""
    nc = g.nc
    ps_ab = g.aux.tile([128, 4, 2], F32, tag="aux", name=f"ps_ab{bi}")
    nc.tensor.matmul(ps_ab, g.e8, gab, start=True, stop=True)
    # A = rstd * gn_scale ; Bb = gn_bias - mean * A
    ab = g.small.tile([128, 4, 2], F32, tag="ab", name=f"ab{bi}")
    tmpc = g.small.tile([128, 4, 1], F32, tag="tmpc", name=f"tmpc{bi}")
    nc.vector.tensor_mul(ab[:, :, 0:1], ps_ab[:, :, 0:1], g.gns_sb[:, :, 0:1])
    nc.vector.tensor_mul(tmpc, ps_ab[:, :, 1:2], ab[:, :, 0:1])
    nc.vector.tensor_sub(ab[:, :, 1:2], g.gnb_sb[:, :, 0:1], tmpc)
    zt = g.zt_p.tile([128, 4, 1024], MM_DT, tag="zt", name=f"zt{bi}")
    for t in range(4):
        nc.vector.tensor_scalar(
            out=zt[:, t, :], in0=xt[:, t, :],
            scalar1=ab[:, t, 0:1], scalar2=ab[:, t, 1:2],
            op0=ALU.mult, op1=ALU.add,
        )
    return zt


def _gv_stage(g, bi, zt):
    """G^T = M'^T Z^T and V' = Z Wv' (natural)."""
    nc = g.nc
    gt = g.gt_p.tile([128, 4, 1024], MM_DT, tag="gt")
    for m in range(4):
        ps = [g.pm.tile([128, 512], F32, tag="pm", name=f"ps_g{bi}_{m}_{h}")
              for h in range(2)]
        for kk in range(4):
            for h in range(2):
                nc.tensor.matmul(
                    ps[h],
                    g.mq_sb[:, kk, m * 128:(m + 1) * 128],
                    zt[:, kk, h * 512:(h + 1) * 512],
                    start=(kk == 0),
                    stop=(kk == 3),
                )
        for h in range(2):
            nc.scalar.copy(gt[:, m, h * 512:(h + 1) * 512], ps[h])
    v = g.v_p.tile([128, 8, 512], MM_DT, tag="v")
    for i in range(8):
        ps = g.pm.tile([128, 512], F32, tag="pm")
        for kk in range(4):
            nc.tensor.matmul(
                ps,
                zt[:, kk, i * 128:(i + 1) * 128],
                g.wvp_sb[:, kk, :],
                start=(kk == 0),
                stop=(kk == 3),
            )
        nc.vector.tensor_copy(v[:, i, :], ps)
    return gt, v


def _u_stage(g, bi, zt):
    """Per-key bias u_j = uvec . z_j  (only when b_qkv != 0)."""
    nc = g.nc
    ps_u = g.aux.tile([128, 8], F32, tag="aux", name=f"ps_u{bi}")
    for j in range(8):
        for kk in range(4):
            nc.tensor.matmul(
                ps_u[:, j:j + 1],
                zt[:, kk, j * 128:(j + 1) * 128],
                g.uv_sb[:, kk:kk + 1],
                start=(kk == 0),
                stop=(kk == 3),
            )
    u_sb = g.small.tile([128, 8], F32, tag="u_sb", name=f"u_sb{bi}")
    nc.vector.tensor_copy(u_sb, ps_u)
    return u_sb


def _phase_st(g, bi, zt, gt, u_sb=None, tr=None):
    """Transposed scores + exp -> ET; optionally interleave the next
    example's transpose groups (dependency-free PE work)."""
    nc = g.nc
    et = g.et_p.tile([128, 8, 1024], MM_DT, tag="et")
    for j in range(8):
        ps = [g.pm.tile([128, 512], F32, tag="pm", name=f"ps_s{bi}_{j}_{h}")
              for h in range(2)]
        for ct in range(4):
            for h in range(2):
                nc.tensor.matmul(
                    ps[h],
                    zt[:, ct, j * 128:(j + 1) * 128],
                    gt[:, ct, h * 512:(h + 1) * 512],
                    start=(ct == 0),
                    stop=(ct == 3),
                )
        for h in range(2):
            if u_sb is None:
                nc.scalar.activation(
                    et[:, j, h * 512:(h + 1) * 512], ps[h], AF.Exp)
            else:
                nc.scalar.activation(
                    et[:, j, h * 512:(h + 1) * 512], ps[h], AF.Exp,
                    bias=u_sb[:, j:j + 1])
        if tr is not None:
            tr(j)
    return et


def _phase_o(g, bi, xn, v, et, mid1=None, mid2=None):
    """O natural (+ denominators), residual, store.  mid1/mid2 emit the
    next example's stats matmuls at points where their DVE inputs are
    already complete (no PE-FIFO stall)."""
    nc = g.nc
    res = g.res_p.tile([128, 8, 512], F32, tag="res")
    out_q = [nc.sync, nc.scalar]
    for i in range(8):
        ps_o = g.pm.tile([128, 512], F32, tag="pm", name=f"ps_o{bi}_{i}")
        ps_s = g.aux.tile([128, 1], F32, tag="aux", name=f"ps_sd{bi}_{i}")
        for j in range(8):
            nc.tensor.matmul(
                ps_o,
                et[:, j, i * 128:(i + 1) * 128],
                v[:, j, :],
                start=(j == 0),
                stop=(j == 7),
            )
            nc.tensor.matmul(
                ps_s,
                et[:, j, i * 128:(i + 1) * 128],
                g.ones_r,
                start=(j == 0),
                stop=(j == 7),
            )
        # b' pre-add into xn (after the transposes/stats read xn)
        nc.vector.tensor_add(xn[:, i, :], xn[:, i, :], g.bres_bc)
        rc = g.small.tile([128, 1], F32, tag="rc", name=f"rc{bi}_{i}")
        nc.vector.reciprocal(rc, ps_s)
        nc.vector.scalar_tensor_tensor(
            out=res[:, i, :], in0=ps_o, scalar=rc,
            in1=xn[:, i, :], op0=ALU.mult, op1=ALU.add,
        )
        out_q[i % 2].dma_start(g.outr[bi, :, i, :], res[:, i, :])
        if i == 1 and mid1 is not None:
            mid1()
        if i == 3 and mid2 is not None:
            mid2()


def build_program(has_u):
    nc = bacc.Bacc("TRN2", target_bir_lowering=False, debug=False)

    x_d = nc.dram_tensor("x", [BPC, N, C], F32, kind="ExternalInput")
    mq_d = nc.dram_tensor("m_qk", [C, C], MM_DT, kind="ExternalInput")
    wvp_d = nc.dram_tensor("w_vp", [C, C], MM_DT, kind="ExternalInput")
    bres_d = nc.dram_tensor("b_res", [C], F32, kind="ExternalInput")
    gns_d = nc.dram_tensor("gn_scale", [C], F32, kind="ExternalInput")
    gnb_d = nc.dram_tensor("gn_bias", [C], F32, kind="ExternalInput")
    if has_u:
        uv_d = nc.dram_tensor("u_vec", [C], F32, kind="ExternalInput")
    out_d = nc.dram_tensor("out", [BPC, N, C], F32, kind="ExternalOutput")

    g = Ctx()
    g.nc = nc
    g.xr = x_d.ap().rearrange("b (i p) c -> b p i c", p=128)
    g.outr = out_d.ap().rearrange("b (i p) c -> b p i c", p=128)

    with tile.TileContext(nc) as tc:
        from contextlib import ExitStack
        with ExitStack() as ctx:
            const = ctx.enter_context(tc.tile_pool(name="const", bufs=1))
            g.pm = ctx.enter_context(tc.tile_pool(name="pm", bufs=6, space=MS.PSUM))
            g.aux = ctx.enter_context(tc.tile_pool(name="aux", bufs=2, space=MS.PSUM))
            g.xn_p = ctx.enter_context(tc.tile_pool(name="xn", bufs=2))
            g.xt_p = ctx.enter_context(tc.tile_pool(name="xtp", bufs=1))
            g.zt_p = ctx.enter_context(tc.tile_pool(name="ztp", bufs=2))
            g.gt_p = ctx.enter_context(tc.tile_pool(name="gtp", bufs=1))
            g.v_p = ctx.enter_context(tc.tile_pool(name="vp", bufs=2))
            g.et_p = ctx.enter_context(tc.tile_pool(name="etp", bufs=2))
            g.res_p = ctx.enter_context(tc.tile_pool(name="resp", bufs=1))
            g.small = ctx.enter_context(tc.tile_pool(name="small", bufs=3))

            # ---- example-0 input DMA first: it is on the critical path
            xn0 = _load_x(g, 0, nq=3)

            # ---- constants ----------------------------------------------
            g.ident = const.tile([128, 128], F32)
            make_identity(nc, g.ident)

            # PE warmup: real matmuls with no DMA dependency, issued while
            # the input DMAs run, so the HAM clock gate reaches K=8/8
            # before the first productive matmul.
            def warm(n, salt=[0]):
                for _ in range(n):
                    salt[0] += 1
                    ps_w = g.pm.tile([128, 512], F32, tag="pm",
                                     name=f"ps_w{salt[0]}")
                    nc.tensor.matmul(ps_w[:, 0:128], g.ident, g.ident,
                                     start=True, stop=True)
            g.warm = warm
            warm(24)

            g.a_pool = const.tile([128, 8], F32)
            nc.gpsimd.memset(g.a_pool, 1.0 / GS)
            nc.gpsimd.affine_select(
                out=g.a_pool, in_=g.a_pool, compare_op=ALU.is_ge, fill=0.0,
                base=0, pattern=[[-GS, 8]], channel_multiplier=1)
            nc.gpsimd.affine_select(
                out=g.a_pool, in_=g.a_pool, compare_op=ALU.is_ge, fill=0.0,
                base=GS - 1, pattern=[[GS, 8]], channel_multiplier=-1)

            g.e8 = const.tile([8, 128], F32)
            nc.gpsimd.memset(g.e8, 1.0)
            nc.gpsimd.affine_select(
                out=g.e8, in_=g.e8, compare_op=ALU.is_ge, fill=0.0,
                base=0, pattern=[[1, 128]], channel_multiplier=-GS)
            nc.gpsimd.affine_select(
                out=g.e8, in_=g.e8, compare_op=ALU.is_ge, fill=0.0,
                base=GS - 1, pattern=[[-1, 128]], channel_multiplier=GS)

            g.ones_r = const.tile([128, 1], MM_DT)
            nc.vector.memset(g.ones_r, 1.0)
            g.eps_c = const.tile([128, 1], F32)
            nc.vector.memset(g.eps_c, EPS)

            g.mq_sb = const.tile([128, 4, C], MM_DT)
            mqr = mq_d.ap().rearrange("(t p) d -> t p d", p=128)
            for t in range(4):
                nc.gpsimd.dma_start(g.mq_sb[:, t, :], mqr[t])
            g.wvp_sb = const.tile([128, 4, C], MM_DT)
            wvr = wvp_d.ap().rearrange("(t p) d -> t p d", p=128)
            for t in range(4):
                nc.gpsimd.dma_start(g.wvp_sb[:, t, :], wvr[t])

            g.gns_sb = const.tile([128, 4, 1], F32)
            nc.sync.dma_start(g.gns_sb[:, :, 0:1],
                              gns_d.ap().rearrange("(t p) -> p t", p=128))
            g.gnb_sb = const.tile([128, 4, 1], F32)
            nc.sync.dma_start(g.gnb_sb[:, :, 0:1],
                              gnb_d.ap().rearrange("(t p) -> p t", p=128))

            def bcast(src_ap):
                return bass.AP(
                    tensor=src_ap.tensor, offset=src_ap.offset,
                    ap=[[0, 128]] + [list(p) for p in src_ap.ap])

            g.bres_bc = const.tile([128, 512], F32)
            nc.gpsimd.dma_start(g.bres_bc, bcast(bres_d.ap()))

            if has_u:
                uvf = const.tile([128, 4], F32)
                nc.sync.dma_start(uvf, uv_d.ap().rearrange("(t p) -> p t", p=128))
                g.uv_sb = const.tile([128, 4], MM_DT)
                nc.vector.tensor_copy(g.uv_sb, uvf)

            # ---- example-0 prologue -------------------------------------
            xt0 = g.xt_p.tile([128, 4, 1024], MM_DT, tag="xt", name="xt0")
            st6_0 = g.small.tile([128, 4, 2, 6], F32, tag="st6", name="st6_0")
            for t in range(4):
                for half in range(2):
                    _tr_group(g, 0, xn0, xt0, st6_0, t, half)
            gab0 = _stats_pool(g, 0, st6_0)
            warm(10)
            zt0 = _stats_norm(g, 0, xt0, gab0)

            # ---- pipelined per-example emission -------------------------
            state = (xn0, zt0)
            nxt = {}
            for bi in range(BPC):
                xn, zt = state
                gt, v = _gv_stage(g, bi, zt)
                u_sb = _u_stage(g, bi, zt) if has_u else None
                tr = None
                if bi + 1 < BPC:
                    xn1 = _load_x(g, bi + 1)
                    xt1 = g.xt_p.tile([128, 4, 1024], MM_DT, tag="xt",
                                      name=f"xt{bi+1}")
                    st6 = g.small.tile([128, 4, 2, 6], F32, tag="st6",
                                       name=f"st6_{bi+1}")

                    def tr(j, xn1=xn1, xt1=xt1, st6=st6, b1=bi + 1):
                        _tr_group(g, b1, xn1, xt1, st6, j // 2, j % 2)
                et = _phase_st(g, bi, zt, gt, u_sb, tr)
                mid1 = mid2 = None
                if bi + 1 < BPC:
                    def mid1(st6=st6, b1=bi + 1):
                        nxt["gab"] = _stats_pool(g, b1, st6)

                    def mid2(xt1=xt1, b1=bi + 1):
                        nxt["zt"] = _stats_norm(g, b1, xt1, nxt["gab"])
                _phase_o(g, bi, xn, v, et, mid1, mid2)
                if bi + 1 < BPC:
                    state = (xn1, nxt["zt"])

    nc.compile()
    return nc


_NC = {}


def _get_nc(has_u):
    if has_u not in _NC:
        _NC[has_u] = build_program(has_u)
    return _NC[has_u]


def kernel(x, t, gn_scale, gn_bias, w_qkv, b_qkv, w_out, b_out):
    import ml_dtypes
    x = np.ascontiguousarray(np.asarray(x, np.float32).reshape(B, N, C))
    w_qkv = np.asarray(w_qkv, np.float32)
    b_qkv = np.asarray(b_qkv, np.float32)
    w_out = np.asarray(w_out, np.float32)
    b_out = np.asarray(b_out, np.float32)
    wq, wk, wv = w_qkv[:, 0:C], w_qkv[:, C:2 * C], w_qkv[:, 2 * C:3 * C]
    bq, bv = b_qkv[0:C], b_qkv[2 * C:3 * C]

    m_qk = ((wq @ wk.T) * ISQ).astype(ml_dtypes.bfloat16)
    w_vp = (wv @ w_out).astype(ml_dtypes.bfloat16)
    b_res = (bv @ w_out + b_out).astype(np.float32)
    u_vec = ((wk @ bq) * ISQ).astype(np.float32)
    has_u = bool(np.any(u_vec != 0.0))

    shared = {
        "m_qk": np.ascontiguousarray(m_qk),
        "w_vp": np.ascontiguousarray(w_vp),
        "b_res": np.ascontiguousarray(b_res),
        "gn_scale": np.ascontiguousarray(np.asarray(gn_scale, np.float32)),
        "gn_bias": np.ascontiguousarray(np.asarray(gn_bias, np.float32)),
    }
    if has_u:
        shared["u_vec"] = np.ascontiguousarray(u_vec)
    in_maps = [
        {"x": x[c * BPC:(c + 1) * BPC], **shared} for c in range(NCORES)
    ]
    nc = _get_nc(has_u)
    res = run_bass_kernel_spmd(nc, in_maps, core_ids=list(range(NCORES)))
    out = np.concatenate([res.results[c]["out"] for c in range(NCORES)], axis=0)
    return out.reshape(B, H, W, C)
